# revision 4
# baseline (speedup 1.0000x reference)
"""KAT rational-group activation kernel for Trainium2 (Bass/Tile), 8-core SPMD.

Computes out = num(x) / den(x) elementwise over x:(4,4096,2048) f32, where
  num(x) = Horner(x, a0..a5)            (numerator coeffs shared everywhere)
  den(x) = Horner(x, [1, |b1..b4|])     (per-group g = channel // 256)

Strategy: shard the sequence dim L across 8 NeuronCores (pure data parallel).
Per core, tiles of [128 positions, 2048 channels] f32 stream through 5 DVE
instructions per tile:
  1. KAT_DEN   (custom, per-group free-dim slice, exact reference Horner order)
  2. reciprocal_approx_fast (stock custom op, ~51 ULP)
  3. KAT_NUMQ  (custom, Horner prefix through a2)
  4. KAT_NUMM  (custom, Horner finish through a0)
  5. tensor_mul (num * recip)
Coefficients are baked as compile-time instruction immediates (3 per op) plus
one [P,1] spilled scalar rides in1 from a tiny replicated coef tensor.
"""

import numpy as np

B, L, D = 4, 4096, 2048
N_CORES = 8
L_SH = L // N_CORES            # 512
ROWS = B * L_SH                # 2048 rows per core shard
P = 128                        # SBUF partitions
N_TILES = ROWS // P            # 16 tiles of [128, D]

_OPS_CACHE = {}


def _register_ops():
    """Define + register the three KAT custom DVE ops (idempotent)."""
    if _OPS_CACHE:
        return _OPS_CACHE

    from concourse import dve_ops
    from concourse.dve_ops import DveOp
    from concourse.dve_spec import (
        C0, C1, C2, C3, One, Spec, Src0, Src1,
        _has_src1, _spill_c3_to_src1, lower,
    )
    from concourse.dve_uop import DveOpSpec

    # den = (((c4*x + c3)*x + c2)*x + c1)*x + 1   [C0..C2 imm, C3 -> in1 spill]
    den_body = _spill_c3_to_src1(
        (((C0 * Src0 + C1) * Src0 + C2) * Src0 + C3) * Src0 + One
    )
    den_ref = lambda in0, in1, s0, s1, imm2: (
        (((s0 * in0.astype(np.float32) + s1) * in0 + imm2) * in0
         + np.asarray(in1, np.float32).reshape(-1, 1)) * in0 + 1.0
    )

    # Q = ((a5*x + a4)*x + a3)*x + a2             [C0..C2 imm, C3 -> in1 spill]
    numq_body = _spill_c3_to_src1(
        ((C0 * Src0 + C1) * Src0 + C2) * Src0 + C3
    )
    numq_ref = lambda in0, in1, s0, s1, imm2: (
        ((s0 * in0.astype(np.float32) + s1) * in0 + imm2) * in0
        + np.asarray(in1, np.float32).reshape(-1, 1)
    )

    # M = (Q*x + a1)*x + a0                        [two full streams]
    numm_body = (Src0 * Src1 + C0) * Src1 + C1
    numm_ref = lambda in0, in1, s0, s1, imm2: (
        (in0.astype(np.float32) * in1 + s0) * in1 + s1
    )

    defs = [
        ("KAT_DEN", den_body, den_ref),
        ("KAT_NUMQ", numq_body, numq_ref),
        ("KAT_NUMM", numm_body, numm_ref),
    ]

    existing = {op.name for op in dve_ops.OPS}
    for i, (name, body, ref) in enumerate(defs):
        if name in existing:
            _OPS_CACHE[name] = next(op for op in dve_ops.OPS if op.name == name)
            continue
        spec = Spec(body=body, reference=ref)
        row = max(dve_ops._SUB_OPCODE_FOR_NAME.values()) + 1
        assert row < 0x20, "custom DVE row field overflow"
        dve_ops._SUB_OPCODE_FOR_NAME[name] = row
        shas = {}
        for ver in ("v3", "v4"):
            uops = lower(spec, ver=ver)
            shas[ver] = DveOpSpec(
                name=name, opcode=row, uops=uops, rd1_en=_has_src1(spec)
            ).sha(ver)
        op = DveOp(name, spec, subdim=False, uops_sha=shas)
        dve_ops.OPS.append(op)
        dve_ops.CUSTOM_DVE_SPECS[name] = spec
        _OPS_CACHE[name] = op
    return _OPS_CACHE


def _build_module(a, c, G):
    """Trace the per-core Bass module. a:(6,) numerator, c:(G,5) |den| coeffs."""
    import concourse.bacc as bacc
    import concourse.mybir as mybir
    from concourse.tile import TileContext

    ops = _register_ops()
    f32 = mybir.dt.float32
    W = D // G  # channels per group

    nc = bacc.Bacc("TRN2", target_bir_lowering=False)
    x = nc.dram_tensor("x", (ROWS, D), f32, kind="ExternalInput")
    coef = nc.dram_tensor("coef", (P, G + 1), f32, kind="ExternalInput")
    y = nc.dram_tensor("y", (ROWS, D), f32, kind="ExternalOutput")

    with TileContext(nc) as tc:
        with tc.tile_pool(name="const", bufs=1) as cpool, \
             tc.tile_pool(name="work", bufs=3) as pool:
            ct = cpool.tile([P, G + 1], f32)
            nc.sync.dma_start(out=ct[:], in_=coef[:, :])
            for i in range(N_TILES):
                r0 = i * P
                xt = pool.tile([P, D], f32, tag="x")
                nc.sync.dma_start(out=xt[:], in_=x[r0:r0 + P, :])

                dent = pool.tile([P, D], f32, tag="den")
                for g in range(G):
                    sl = slice(g * W, (g + 1) * W)
                    nc.vector._custom_dve(
                        ops["KAT_DEN"],
                        out=dent[:, sl], in0=xt[:, sl], in1=ct[:, g:g + 1],
                        s0=float(c[g, 4]), s1=float(c[g, 3]), imm2=float(c[g, 2]),
                    )
                rt = pool.tile([P, D], f32, tag="r")
                nc.vector.reciprocal_approx_fast(out=rt[:], in_=dent[:])

                qt = pool.tile([P, D], f32, tag="q")
                nc.vector._custom_dve(
                    ops["KAT_NUMQ"],
                    out=qt[:], in0=xt[:], in1=ct[:, G:G + 1],
                    s0=float(a[5]), s1=float(a[4]), imm2=float(a[3]),
                )
                mt = pool.tile([P, D], f32, tag="m")
                nc.vector._custom_dve(
                    ops["KAT_NUMM"],
                    out=mt[:], in0=qt[:], in1=xt[:],
                    s0=float(a[1]), s1=float(a[0]),
                )
                ot = pool.tile([P, D], f32, tag="o")
                nc.vector.tensor_mul(ot[:], mt[:], rt[:])
                nc.sync.dma_start(out=y[r0:r0 + P, :], in_=ot[:])
    nc.compile()
    return nc


def kernel(x, weight_numerator, weight_denominator, num_groups):
    from concourse import bass_utils

    x = np.ascontiguousarray(np.asarray(x, dtype=np.float32))
    a = np.asarray(weight_numerator, np.float32).reshape(-1)          # (6,)
    wd = np.asarray(weight_denominator, np.float32)                   # (G,4)
    G = int(num_groups)
    c = np.abs(np.concatenate([np.ones((G, 1), np.float32), wd], axis=1))

    nc = _build_module(a, c, G)

    coef_arr = np.zeros((P, G + 1), np.float32)
    coef_arr[:, :G] = c[:, 1][None, :]     # per-group c1 (spilled C3 of KAT_DEN)
    coef_arr[:, G] = a[2]                  # a2 (spilled C3 of KAT_NUMQ)

    xr = x.reshape(B, N_CORES, L_SH, D)
    in_maps = [
        {"x": np.ascontiguousarray(xr[:, core]).reshape(ROWS, D),
         "coef": coef_arr}
        for core in range(N_CORES)
    ]
    res = bass_utils.run_bass_kernel_spmd(nc, in_maps, core_ids=list(range(N_CORES)))

    out = np.empty((B, N_CORES, L_SH, D), np.float32)
    for core in range(N_CORES):
        out[:, core] = res.results[core]["y"].reshape(B, L_SH, D)
    return out.reshape(B, L, D)
